# revision 41
# baseline (speedup 1.0000x reference)
"""Trainium2 Bass kernel for the scalar-input GRU (B=512, T=128, H=512) + ReLU/Linear head.

Data-parallel over batch across 8 NeuronCores (64 rows each); per core the 64
rows run as W=2 interleaved waves of 32 so one wave's gate algebra overlaps the
other wave's matmuls.

v4: latency-oriented rewrite around fp8 DoubleRow matmuls and a split h-state.

- All W_hh matmuls are fp8e4 DoubleRow (0.5 cyc/row, K=256/matmul). W_hh is
  scaled x64 into fp8's normal range; sigmoid/tanh unscale via ACT's free
  `scale=1/64`.
- The h state feeding PE is SPLIT by linearity: W@h = W@u8 + W@q8 with
  u8 = fp8(cv*n) and q8 = fp8(z*s).  q8 is ready mid-step (off-chain, Pool);
  u8 is the ONLY post-tanh op on the critical path.  The bf16 master state
  s = u16 + q16 stays exact, so fp8 noise perturbs only preactivations and
  does not accumulate (validated: rel err ~8.8e-3 vs the 2e-2 gate).
- Each (gate, wave) PSUM bank runs ONE accumulation group per step:
  [aug, q8-DR, q8-DR] x4 j-chunks early (open), then 8 u8-DRs late (stop) --
  so early matmuls never queue behind the u8-gated ones and sigmoid(r) waits
  exactly one group-stop sem.
- n-gate: m = (P_n * 1/64) * r in ONE fused scalar_tensor_tensor op; gx_n + m
  accumulate on PE (4 aug + 1 identity matmul, one group) into psn2; tanh
  reads psn2.
"""

import sys

sys.path.insert(0, "/opt/trn_rl_repo")

import numpy as np

import concourse.bacc as bacc
import concourse.bass as bass
import concourse.mybir as mybir
import concourse.tile as tile
from concourse.bass_utils import run_bass_kernel_spmd
from concourse.masks import make_identity

N_CORES = 8
B_FULL, T_FULL, H = 512, 128, 512
B = B_FULL // N_CORES  # 64 batch rows per core
W = 2  # waves per core
BW = B // W  # 32 rows per wave
G3 = 3 * H  # 1536
NK = H // 128  # 4 contraction chunks
NJ = 4  # j-chunks per gate (H/128)
F32 = mybir.dt.float32
BF16 = mybir.dt.bfloat16
FP8 = mybir.dt.float8e4
AF = mybir.ActivationFunctionType
ALU = mybir.AluOpType
DR = mybir.MatmulPerfMode.DoubleRow
WSC = 64.0  # fp8 weight scale


def build_nc(T: int = T_FULL) -> bass.Bass:
    nc = bacc.Bacc("TRN2", target_bir_lowering=False, debug=False)

    x_d = nc.dram_tensor("x", [B, T], F32, kind="ExternalInput")
    whh_d = nc.dram_tensor("w_hh", [G3, H], F32, kind="ExternalInput")
    wih_d = nc.dram_tensor("w_ih", [G3, 1], F32, kind="ExternalInput")
    bih_d = nc.dram_tensor("b_ih", [G3], F32, kind="ExternalInput")
    bhh_d = nc.dram_tensor("b_hh", [G3], F32, kind="ExternalInput")
    fcw_d = nc.dram_tensor("fc_w", [1, H], F32, kind="ExternalInput")
    fcb_d = nc.dram_tensor("fc_b", [1], F32, kind="ExternalInput")
    out_d = nc.dram_tensor("out", [B, 1], F32, kind="ExternalOutput")

    with tile.TileContext(nc) as tc:
        _body(tc, T, x_d, whh_d, wih_d, bih_d, bhh_d, fcw_d, fcb_d, out_d)
    nc.compile()
    return nc


def _body(tc, T, x_d, whh_d, wih_d, bih_d, bhh_d, fcw_d, fcb_d, out_d):
    nc = tc.nc
    with (
        tc.tile_pool(name="const", bufs=1) as cpool,
        tc.tile_pool(name="state", bufs=3) as spool,
        tc.tile_pool(name="work", bufs=3) as wpool,
        tc.tile_pool(name="psmain", bufs=2, space="PSUM") as ppool,
    ):
        # ---- one-time prep ----
        wstage = cpool.tile([128, (G3 // 128) * H], F32)
        for cg in range(12):
            nc.sync.dma_start(
                out=wstage[:, cg * H : (cg + 1) * H],
                in_=whh_d[cg * 128 : (cg + 1) * 128, :],
            )

        ident128 = cpool.tile([128, 128], F32)
        make_identity(nc, ident128)
        identb = cpool.tile([128, 128], BF16)
        nc.vector.tensor_copy(identb[:, :], ident128[:, :])
        ident64 = cpool.tile([64, 64], F32)
        make_identity(nc, ident64)

        x_sb = cpool.tile([B, T], F32)
        nc.scalar.dma_start(out=x_sb[:, :], in_=x_d[:, :])
        wi12 = cpool.tile([12, 128], F32)
        nc.scalar.dma_start(
            out=wi12[:, :], in_=wih_d[:, :].rearrange("(p c) one -> p (c one)", p=12)
        )
        bs12 = cpool.tile([12, 128], F32)
        nc.gpsimd.dma_start(
            out=bs12[:, :], in_=bhh_d[None, :].rearrange("one (p c) -> (one p) c", p=12)
        )
        bi12 = cpool.tile([12, 128], F32)
        nc.gpsimd.dma_start(
            out=bi12[:, :], in_=bih_d[None, :].rearrange("one (p c) -> (one p) c", p=12)
        )
        fcwf = cpool.tile([128, NK], F32)
        nc.scalar.dma_start(
            out=fcwf[:, :],
            in_=fcw_d[:, :]
            .rearrange("one (k p) -> one k p", p=128)
            .transpose([2, 0, 1])
            .rearrange("p one k -> p (one k)"),
        )
        fcbf = cpool.tile([1, 1], F32)
        nc.gpsimd.dma_start(out=fcbf[:, :], in_=fcb_d[None, :])
        onesf = cpool.tile([1, B], F32)
        nc.gpsimd.memset(onesf[:, :], 1.0)

        # bsum = b_hh + b_ih on the r/z rows (rows 0:8); rows 8:12 stay b_hh_n
        nc.vector.tensor_add(bs12[0:8, :], bs12[0:8, :], bi12[0:8, :])
        # scaled copies for the x64 PSUM convention of P_r/P_z/P_n
        wi64 = cpool.tile([12, 128], F32)
        nc.vector.tensor_scalar_mul(wi64[:, :], wi12[:, :], WSC)
        bs64 = cpool.tile([12, 128], F32)
        nc.vector.tensor_scalar_mul(bs64[:, :], bs12[:, :], WSC)

        # bf16 casts of (wi64, bs64, bih, wi) into one shared tile, one DMA
        hi_all = cpool.tile([12, 512], BF16)
        nc.gpsimd.tensor_copy(hi_all[:, 0:128], wi64[:, :])
        nc.gpsimd.tensor_copy(hi_all[:, 128:256], bs64[:, :])
        nc.gpsimd.tensor_copy(hi_all[:, 256:384], bi12[:, :])
        nc.gpsimd.tensor_copy(hi_all[:, 384:512], wi12[:, :])
        scr_d = nc.dram_tensor("scr_aug", [4, 12, 128], BF16, kind="Internal")
        nc.sync.dma_start(
            out=scr_d[:, :, :].transpose([1, 0, 2]),
            in_=hi_all[:, :].rearrange("p (k c) -> p k c", k=4),
        )

        # xaug rows: (x, 1)
        xaug = cpool.tile([2, T * B], BF16)
        nc.gpsimd.memset(xaug[:, :].bitcast(mybir.dt.uint32), 0x3F803F80)
        ones1 = cpool.tile([1, B], BF16)
        nc.gpsimd.memset(ones1[:, :], 1.0)

        xt_ps = ppool.tile([T, B], F32, tag="psPREP", bufs=2, name="xt_ps")
        nc.tensor.transpose(xt_ps[:, :], x_sb[:, :], ident64)
        xt_b = cpool.tile([T, B], BF16)
        nc.vector.tensor_copy(xt_b[:, :], xt_ps[:, :])
        xt_scr = nc.dram_tensor("xt_scr", [T, B], BF16, kind="Internal")
        nc.scalar.dma_start(out=xt_scr[:, :], in_=xt_b[:, :])
        nc.sync.dma_start(
            out=xaug[0:1, :], in_=xt_scr[:, :].rearrange("p c -> (p c)")[None, :]
        )

        # Stationary aug tiles:
        #   AUG   [2, 2H]: (wi*64, bsum*64) rows for r,z; rhs = xaug (x, 1)
        #   AUGNB [1, H]:  (b_hh_n*64); rhs = ones1
        #   AUGG  [2, H]:  (wi_n, bih_n) UNSCALED for psn2 = gx_n; rhs = xaug
        AUG = cpool.tile([2, 2 * H], BF16)
        AUGNB = cpool.tile([1, H], BF16)
        AUGG = cpool.tile([2, H], BF16)

        def row_dma(q, dst, r, kind, p0, p1):
            q.dma_start(
                out=dst[r : r + 1, :],
                in_=scr_d[kind, p0:p1, :].rearrange("p c -> (p c)")[None, :],
            )

        # kinds: 0 = wi*64, 1 = bs*64 (b_hh_n*64 on rows 8:12), 2 = bih, 3 = wi
        row_dma(nc.sync, AUG, 0, 0, 0, 8)
        row_dma(nc.sync, AUG, 1, 1, 0, 8)
        row_dma(nc.scalar, AUGNB, 0, 1, 8, 12)
        row_dma(nc.scalar, AUGG, 0, 3, 8, 12)
        row_dma(nc.scalar, AUGG, 1, 2, 8, 12)

        # w_hh.T in fp8, x64, DoubleRow-packed:
        #   block bb = c*2 + half  (c = global j-chunk 0..11, half = k-half)
        #   wT8[p, bb*256 + i*128 + j] = 64 * w_hh[c*128+j, (2*half+i)*128 + p]
        wT8 = cpool.tile([128, 24 * 256], FP8)
        prep_tags = ["psR0", "psZ0", "psN0", "psR1", "psZ1", "psN1"]
        pi = 0
        for c in range(12):
            for k in range(NK):
                tp = ppool.tile([128, 128], F32, tag=prep_tags[pi % 6], bufs=1,
                                name=f"wprep_{c}_{k}")
                nc.tensor.transpose(
                    tp[:, :], wstage[:, c * H + k * 128 : c * H + (k + 1) * 128],
                    ident128,
                )
                bb = c * 2 + k // 2
                dst = wT8[:, bb * 256 + (k % 2) * 128 : bb * 256 + (k % 2) * 128 + 128]
                if pi % 2 == 0:
                    nc.vector.tensor_scalar_mul(dst, tp[:, :], WSC)
                else:
                    nc.scalar.activation(dst, tp[:, :], AF.Copy, scale=WSC)
                pi += 1

        # state init: u8/q8 fp8 (PE inputs), s bf16 (master h), per wave
        u8T, q8T, sT = [], [], []
        for w in range(W):
            u0 = spool.tile([128, NK * BW], FP8, tag=f"u8{w}", name=f"u8{w}_init")
            nc.gpsimd.memset(u0[:, :], 0.0)
            u8T.append(u0)
            q0 = spool.tile([128, NK * BW], FP8, tag=f"q8{w}", name=f"q8{w}_init")
            nc.gpsimd.memset(q0[:, :], 0.0)
            q8T.append(q0)
            s0 = spool.tile([128, NK * BW], BF16, tag=f"s{w}", name=f"s{w}_init")
            nc.gpsimd.memset(s0[:, :], 0.0)
            sT.append(s0)

        def drview(tile_, half):
            # [128, 2, BW] moving view: k-tiles (2*half, 2*half+1)
            return tile_[:, half * 2 * BW : (half + 1) * 2 * BW].rearrange(
                "p (i b) -> p i b", i=2
            )

        GBASE = {0: 0, 1: 4, 2: 8}  # gate -> global j-chunk base (r, z, n)

        def emit_pe_early(w, t, ps, q8t, gates=(0, 2, 1)):
            # Open one accumulation group per gate bank: augs + q8 DRs for all
            # four j-chunks.  start=True only on the first matmul of the bank
            # (ZERO_REGION covers the whole bank; later writes zero via the
            # pending mark).  The group stays open; u8 DRs close it later.
            psr, psz, psn, psn2 = ps
            xs = xaug[0:2, t * B + w * BW : t * B + (w + 1) * BW]
            os = ones1[0:1, w * BW : (w + 1) * BW]
            for g in gates:  # r, n, z
                pst = (psr, psz, psn)[g]
                for jc in range(NJ):
                    po = pst[:, jc * BW : (jc + 1) * BW]
                    if g < 2:
                        nc.tensor.matmul(
                            po, AUG[0:2, g * H + jc * 128 : g * H + (jc + 1) * 128],
                            xs, start=(jc == 0), stop=False,
                        )
                    else:
                        nc.tensor.matmul(
                            po, AUGNB[0:1, jc * 128 : (jc + 1) * 128],
                            os, start=(jc == 0), stop=False,
                        )
                    c = GBASE[g] + jc
                    for half in range(2):
                        bb = c * 2 + half
                        nc.tensor.matmul(
                            po,
                            wT8[:, bb * 256 : (bb + 1) * 256].rearrange(
                                "p (i j) -> p i j", i=2
                            ),
                            drview(q8t, half),
                            start=False, stop=False,
                            perf_mode=DR,
                        )

        def emit_pe_late(w, t, ps, u8t, gates=(0, 2, 1)):
            # u8-gated DRs close each gate bank's group (r first).
            psr, psz, psn, psn2 = ps
            for g in gates:
                pst = (psr, psz, psn)[g]
                for jc in range(NJ):
                    po = pst[:, jc * BW : (jc + 1) * BW]
                    c = GBASE[g] + jc
                    for half in range(2):
                        bb = c * 2 + half
                        nc.tensor.matmul(
                            po,
                            wT8[:, bb * 256 : (bb + 1) * 256].rearrange(
                                "p (i j) -> p i j", i=2
                            ),
                            drview(u8t, half),
                            start=False,
                            stop=(jc == NJ - 1 and half == 1),
                            perf_mode=DR,
                        )

        def emit_psn2_augs(w, t, ps):
            # gx_n into psn2; first part of the single psn2 group (identity
            # matmul with m closes it in emit_gpairs)
            psr, psz, psn, psn2 = ps
            xs = xaug[0:2, t * B + w * BW : t * B + (w + 1) * BW]
            for jc in range(NJ):
                nc.tensor.matmul(
                    psn2[:, jc * BW : (jc + 1) * BW],
                    AUGG[0:2, jc * 128 : (jc + 1) * 128],
                    xs, start=(jc == 0), stop=False,
                )

        def emit_gpairs(w, t, ps, st):
            psr, psz, psn, psn2 = ps
            nc.tensor.matmul(
                psn2[:, 0 : 4 * BW], identb[:, :], st["m"][:, 0 : 4 * BW],
                start=False, stop=True,
            )

        def emit_sigr(w, t, ps, st):
            psr, psz, psn, psn2 = ps
            if st.get("rz_t") != t:
                st["rz"] = wpool.tile([128, 8 * BW], BF16, tag=f"rz{w}",
                                      name=f"rz{w}_{t}")
                st["rz_t"] = t
            nc.scalar.activation(st["rz"][:, 0 : 4 * BW], psr[:, :], AF.Sigmoid,
                                 scale=1.0 / WSC)

        def emit_sigz(w, t, ps, st):
            psr, psz, psn, psn2 = ps
            if st.get("rz_t") != t:
                st["rz"] = wpool.tile([128, 8 * BW], BF16, tag=f"rz{w}",
                                      name=f"rz{w}_{t}")
                st["rz_t"] = t
            nc.scalar.activation(st["rz"][:, 4 * BW : 8 * BW], psz[:, :], AF.Sigmoid,
                                 scale=1.0 / WSC)

        def emit_m(w, t, ps, st):
            # m = (P_n/64) * r stays on DVE: GPSIMD cannot read PSUM on
            # real hardware (compile fails), even though the cost model
            # would price it lower there
            psr, psz, psn, psn2 = ps
            rz = st["rz"]
            m = wpool.tile([128, 4 * BW], BF16, tag=f"m{w}", name=f"m{w}_{t}")
            nc.vector.scalar_tensor_tensor(
                m[:, :], psn[:, :], 1.0 / WSC, rz[:, 0 : 4 * BW],
                ALU.mult, ALU.mult,
            )
            st["m"] = m

        def emit_cv(w, t, st):
            rz = st["rz"]
            cv = wpool.tile([128, 4 * BW], BF16, tag=f"cv{w}", name=f"cv{w}_{t}")
            nc.vector.tensor_scalar(cv[:, :], rz[:, 4 * BW : 8 * BW], 1.0, -1.0,
                                    ALU.subtract, ALU.mult)
            st["cv"] = cv

        dly = cpool.tile([128, 256], BF16)

        def emit_pool_delay(t):
            # ~300ns Pool-queue spacer: q8_1 must land AFTER u8_0's sem so the
            # scheduler statically orders next step's w1-earlies behind the
            # chain-critical w0 lates (but before late1's gate)
            nc.gpsimd.memset(dly[:, 0:64], 0.0)

        def emit_q8(w, t, st):
            # q8 = fp8(z * s_old) for PE; both waves' q8 run on Pool BEFORE
            # the q16s so next step's early matmuls unblock as soon as
            # possible
            rz = st["rz"]
            q8n = spool.tile([128, NK * BW], FP8, tag=f"q8{w}", name=f"q8{w}_{t}")
            nc.gpsimd.tensor_tensor(q8n[:, :], rz[:, 4 * BW : 8 * BW], sT[w][:, :],
                                    ALU.mult)
            st["q8n"] = q8n

        def emit_q16(w, t, st, eng=None):
            # q16 = bf16 version for the exact master state.  Wave 0's runs
            # on DVE (2x mode); wave 1's runs on Pool BEFORE q8_1, which
            # delays q8_1 just enough that next step's w1-earlies are
            # statically ordered AFTER the chain-critical w0 lates.
            rz = st["rz"]
            q16 = wpool.tile([128, 4 * BW], BF16, tag=f"q16{w}", name=f"q16{w}_{t}")
            (eng or nc.vector).tensor_tensor(q16[:, :], rz[:, 4 * BW : 8 * BW],
                                             sT[w][:, :], ALU.mult)
            st["q16"] = q16

        def emit_tanh(w, t, ps, st):
            psr, psz, psn, psn2 = ps
            n = wpool.tile([128, 4 * BW], BF16, tag=f"n{w}", name=f"n{w}_{t}")
            nc.scalar.activation(n[:, :], psn2[:, :], AF.Tanh)
            st["n"] = n

        def emit_tail_u(w, t, st):
            n, cv = st["n"], st["cv"]
            # u8 first: it is the ONLY op the next step's matmuls wait on
            u8n = spool.tile([128, NK * BW], FP8, tag=f"u8{w}", name=f"u8{w}_{t}")
            nc.vector.tensor_tensor(u8n[:, :], cv[:, :], n[:, :], ALU.mult)
            u16 = wpool.tile([128, 4 * BW], BF16, tag=f"u16{w}", name=f"u16{w}_{t}")
            nc.vector.tensor_tensor(u16[:, :], cv[:, :], n[:, :], ALU.mult)
            st["u16"] = u16
            u8T[w] = u8n

        def emit_s(w, t, st):
            sn = spool.tile([128, NK * BW], BF16, tag=f"s{w}", name=f"s{w}_{t}")
            nc.vector.tensor_tensor(sn[:, :], st["u16"][:, :], st["q16"][:, :],
                                    ALU.add)
            sT[w] = sn

        def alloc_ps(t):
            out = []
            for w in range(W):
                psr = ppool.tile([128, 4 * BW], F32, tag=f"psR{w}", bufs=1,
                                 name=f"psr{w}_{t}")
                psz = ppool.tile([128, 4 * BW], F32, tag=f"psZ{w}", bufs=1,
                                 name=f"psz{w}_{t}")
                psn = ppool.tile([128, 4 * BW], F32, tag=f"psN{w}", bufs=1,
                                 name=f"psn{w}_{t}")
                psn2 = ppool.tile([128, 4 * BW], F32, tag="psPREP", bufs=2,
                                  name=f"psn2_{w}_{t}")
                out.append((psr, psz, psn, psn2))
            return out

        # ---- the recurrence, fully unrolled, 2 waves interleaved ----
        # PE order per step: earlies(0) [prev iter] | late0 augs0 earlies1
        # late1 ... ident0 augs1 earlies0(t+1) ident1
        sts = [{}, {}]
        pss = alloc_ps(0)
        emit_pe_early(0, 0, pss[0], q8T[0])
        for t in range(T):
            pss_next = alloc_ps(t + 1) if t + 1 < T else None
            emit_pe_early(1, t, pss[1], q8T[1], gates=(0,))
            with tc.high_priority():
                emit_pe_late(0, t, pss[0], u8T[0])
            emit_psn2_augs(0, t, pss[0])
            # n/z earlies of wave 1 sit AFTER the chain-critical w0 lates in
            # priority so their drain cannot delay sigmoid(r0)
            emit_pe_early(1, t, pss[1], q8T[1], gates=(2, 1))
            emit_pe_late(1, t, pss[1], u8T[1])
            with tc.high_priority():
                emit_sigr(0, t, pss[0], sts[0])
            emit_sigz(0, t, pss[0], sts[0])
            with tc.high_priority():
                emit_m(0, t, pss[0], sts[0])
            emit_cv(0, t, sts[0])
            emit_q16(0, t, sts[0])
            emit_q8(0, t, sts[0])
            emit_sigr(1, t, pss[1], sts[1])
            with tc.high_priority():
                emit_gpairs(0, t, pss[0], sts[0])
                emit_tanh(0, t, pss[0], sts[0])
            emit_sigz(1, t, pss[1], sts[1])
            emit_m(1, t, pss[1], sts[1])
            emit_pool_delay(t)
            emit_q8(1, t, sts[1])
            emit_psn2_augs(1, t, pss[1])
            if pss_next is not None:
                emit_pe_early(0, t + 1, pss_next[0], sts[0]["q8n"])
            with tc.high_priority():
                emit_tail_u(0, t, sts[0])
            emit_cv(1, t, sts[1])
            emit_q16(1, t, sts[1])
            emit_gpairs(1, t, pss[1], sts[1])
            emit_tanh(1, t, pss[1], sts[1])
            emit_tail_u(1, t, sts[1])
            emit_s(0, t, sts[0])
            emit_s(1, t, sts[1])
            q8T[0] = sts[0]["q8n"]
            q8T[1] = sts[1]["q8n"]
            pss = pss_next

        # ---- head: out = relu(h) @ fc_w.T + fc_b ----
        pso = ppool.tile([B, 1], F32, tag="psPREP", bufs=2, name="ps_fc")
        for w in range(W):
            reluh = wpool.tile([128, NK * BW], F32, tag=f"relu{w}", name=f"relu{w}")
            nc.scalar.activation(reluh[:, :], sT[w][:, :], AF.Relu)
            po = pso[w * BW : (w + 1) * BW, :]
            nc.tensor.matmul(
                po, onesf[:, 0:BW], fcbf[0:1, 0:1], start=True, stop=False
            )
            for k in range(NK):
                nc.tensor.matmul(
                    po,
                    reluh[:, k * BW : (k + 1) * BW],
                    fcwf[:, k : k + 1],
                    start=False, stop=(k == NK - 1),
                )
        outw = wpool.tile([B, 1], F32, tag="outw", name="out_sb")
        nc.vector.tensor_copy(outw[:, :], pso[:, :])
        nc.sync.dma_start(out=out_d[:, :], in_=outw[:, :])


_NC_CACHE: dict[int, bass.Bass] = {}


def _get_nc(T: int = T_FULL) -> bass.Bass:
    if T not in _NC_CACHE:
        _NC_CACHE[T] = build_nc(T)
    return _NC_CACHE[T]


def kernel(x, w_ih, w_hh, b_ih, b_hh, fc_w, fc_b, _trace=False, _tmpdir=None):
    x = np.ascontiguousarray(np.asarray(x, dtype=np.float32))
    nc = _get_nc(x.shape[1])
    shared = {
        "w_hh": np.ascontiguousarray(np.asarray(w_hh, np.float32)),
        "w_ih": np.ascontiguousarray(np.asarray(w_ih, np.float32)),
        "b_ih": np.ascontiguousarray(np.asarray(b_ih, np.float32)),
        "b_hh": np.ascontiguousarray(np.asarray(b_hh, np.float32)),
        "fc_w": np.ascontiguousarray(np.asarray(fc_w, np.float32)),
        "fc_b": np.ascontiguousarray(np.asarray(fc_b, np.float32)),
    }
    in_maps = [{"x": x[c * B : (c + 1) * B], **shared} for c in range(N_CORES)]
    res = run_bass_kernel_spmd(
        nc, in_maps, list(range(N_CORES)), trace=_trace, tmpdir=_tmpdir
    )
    out = np.concatenate([res.results[c]["out"] for c in range(N_CORES)], axis=0)
    if _trace:
        return out, res
    return out


# revision 42
# speedup vs baseline: 1.0786x; 1.0786x over previous
"""Trainium2 Bass kernel for the scalar-input GRU (B=512, T=128, H=512) + ReLU/Linear head.

Data-parallel over batch across 8 NeuronCores (64 rows each); per core the 64
rows run as W=2 interleaved waves of 32 so one wave's gate algebra overlaps the
other wave's matmuls.

v4: latency-oriented rewrite around fp8 DoubleRow matmuls and a split h-state.

- All W_hh matmuls are fp8e4 DoubleRow (0.5 cyc/row, K=256/matmul). W_hh is
  scaled x64 into fp8's normal range; sigmoid/tanh unscale via ACT's free
  `scale=1/64`.
- The h state feeding PE is SPLIT by linearity: W@h = W@u8 + W@q8 with
  u8 = fp8(cv*n) and q8 = fp8(z*s).  q8 is ready mid-step (off-chain, Pool);
  u8 is the ONLY post-tanh op on the critical path.  The bf16 master state
  s = u16 + q16 stays exact, so fp8 noise perturbs only preactivations and
  does not accumulate (validated: rel err ~8.8e-3 vs the 2e-2 gate).
- Each (gate, wave) PSUM bank runs ONE accumulation group per step:
  [aug, q8-DR, q8-DR] x4 j-chunks early (open), then 8 u8-DRs late (stop) --
  so early matmuls never queue behind the u8-gated ones and sigmoid(r) waits
  exactly one group-stop sem.
- n-gate: m = (P_n * 1/64) * r in ONE fused scalar_tensor_tensor op; gx_n + m
  accumulate on PE (4 aug + 1 identity matmul, one group) into psn2; tanh
  reads psn2.
"""

import sys

sys.path.insert(0, "/opt/trn_rl_repo")

import numpy as np

import concourse.bacc as bacc
import concourse.bass as bass
import concourse.mybir as mybir
import concourse.tile as tile
from concourse.bass_utils import run_bass_kernel_spmd
from concourse.masks import make_identity

N_CORES = 8
B_FULL, T_FULL, H = 512, 128, 512
B = B_FULL // N_CORES  # 64 batch rows per core
W = 2  # waves per core
BW = B // W  # 32 rows per wave
G3 = 3 * H  # 1536
NK = H // 128  # 4 contraction chunks
NJ = 4  # j-chunks per gate (H/128)
F32 = mybir.dt.float32
BF16 = mybir.dt.bfloat16
FP8 = mybir.dt.float8e4
AF = mybir.ActivationFunctionType
ALU = mybir.AluOpType
DR = mybir.MatmulPerfMode.DoubleRow
WSC = 64.0  # fp8 weight scale


def build_nc(T: int = T_FULL) -> bass.Bass:
    nc = bacc.Bacc("TRN2", target_bir_lowering=False, debug=False)

    x_d = nc.dram_tensor("x", [B, T], F32, kind="ExternalInput")
    whh_d = nc.dram_tensor("w_hh", [G3, H], F32, kind="ExternalInput")
    wih_d = nc.dram_tensor("w_ih", [G3, 1], F32, kind="ExternalInput")
    bih_d = nc.dram_tensor("b_ih", [G3], F32, kind="ExternalInput")
    bhh_d = nc.dram_tensor("b_hh", [G3], F32, kind="ExternalInput")
    fcw_d = nc.dram_tensor("fc_w", [1, H], F32, kind="ExternalInput")
    fcb_d = nc.dram_tensor("fc_b", [1], F32, kind="ExternalInput")
    out_d = nc.dram_tensor("out", [B, 1], F32, kind="ExternalOutput")

    with tile.TileContext(nc) as tc:
        _body(tc, T, x_d, whh_d, wih_d, bih_d, bhh_d, fcw_d, fcb_d, out_d)
    nc.compile()
    return nc


def _body(tc, T, x_d, whh_d, wih_d, bih_d, bhh_d, fcw_d, fcb_d, out_d):
    nc = tc.nc
    with (
        tc.tile_pool(name="const", bufs=1) as cpool,
        tc.tile_pool(name="state", bufs=3) as spool,
        tc.tile_pool(name="work", bufs=3) as wpool,
        tc.tile_pool(name="psmain", bufs=2, space="PSUM") as ppool,
    ):
        # ---- one-time prep ----
        wstage = cpool.tile([128, (G3 // 128) * H], F32)
        for cg in range(12):
            nc.sync.dma_start(
                out=wstage[:, cg * H : (cg + 1) * H],
                in_=whh_d[cg * 128 : (cg + 1) * 128, :],
            )

        ident128 = cpool.tile([128, 128], F32)
        make_identity(nc, ident128)
        identb = cpool.tile([128, 128], BF16)
        nc.vector.tensor_copy(identb[:, :], ident128[:, :])
        ident64 = cpool.tile([64, 64], F32)
        make_identity(nc, ident64)

        x_sb = cpool.tile([B, T], F32)
        nc.scalar.dma_start(out=x_sb[:, :], in_=x_d[:, :])
        wi12 = cpool.tile([12, 128], F32)
        nc.scalar.dma_start(
            out=wi12[:, :], in_=wih_d[:, :].rearrange("(p c) one -> p (c one)", p=12)
        )
        bs12 = cpool.tile([12, 128], F32)
        nc.gpsimd.dma_start(
            out=bs12[:, :], in_=bhh_d[None, :].rearrange("one (p c) -> (one p) c", p=12)
        )
        bi12 = cpool.tile([12, 128], F32)
        nc.gpsimd.dma_start(
            out=bi12[:, :], in_=bih_d[None, :].rearrange("one (p c) -> (one p) c", p=12)
        )
        fcwf = cpool.tile([128, NK], F32)
        nc.scalar.dma_start(
            out=fcwf[:, :],
            in_=fcw_d[:, :]
            .rearrange("one (k p) -> one k p", p=128)
            .transpose([2, 0, 1])
            .rearrange("p one k -> p (one k)"),
        )
        fcbf = cpool.tile([1, 1], F32)
        nc.gpsimd.dma_start(out=fcbf[:, :], in_=fcb_d[None, :])
        onesf = cpool.tile([1, B], F32)
        nc.gpsimd.memset(onesf[:, :], 1.0)

        # bsum = b_hh + b_ih on the r/z rows (rows 0:8); rows 8:12 stay b_hh_n
        nc.vector.tensor_add(bs12[0:8, :], bs12[0:8, :], bi12[0:8, :])
        # scaled copies for the x64 PSUM convention of P_r/P_z/P_n
        wi64 = cpool.tile([12, 128], F32)
        nc.vector.tensor_scalar_mul(wi64[:, :], wi12[:, :], WSC)
        bs64 = cpool.tile([12, 128], F32)
        nc.vector.tensor_scalar_mul(bs64[:, :], bs12[:, :], WSC)

        # bf16 casts of (wi64, bs64, bih, wi) into one shared tile, one DMA
        hi_all = cpool.tile([12, 512], BF16)
        nc.gpsimd.tensor_copy(hi_all[:, 0:128], wi64[:, :])
        nc.gpsimd.tensor_copy(hi_all[:, 128:256], bs64[:, :])
        nc.gpsimd.tensor_copy(hi_all[:, 256:384], bi12[:, :])
        nc.gpsimd.tensor_copy(hi_all[:, 384:512], wi12[:, :])
        scr_d = nc.dram_tensor("scr_aug", [4, 12, 128], BF16, kind="Internal")
        nc.sync.dma_start(
            out=scr_d[:, :, :].transpose([1, 0, 2]),
            in_=hi_all[:, :].rearrange("p (k c) -> p k c", k=4),
        )

        # xaug rows: (x, 1)
        xaug = cpool.tile([2, T * B], BF16)
        nc.gpsimd.memset(xaug[:, :].bitcast(mybir.dt.uint32), 0x3F803F80)
        ones1 = cpool.tile([1, B], BF16)
        nc.gpsimd.memset(ones1[:, :], 1.0)

        xt_ps = ppool.tile([T, B], F32, tag="psPREP", bufs=2, name="xt_ps")
        nc.tensor.transpose(xt_ps[:, :], x_sb[:, :], ident64)
        xt_b = cpool.tile([T, B], BF16)
        nc.vector.tensor_copy(xt_b[:, :], xt_ps[:, :])
        xt_scr = nc.dram_tensor("xt_scr", [T, B], BF16, kind="Internal")
        nc.scalar.dma_start(out=xt_scr[:, :], in_=xt_b[:, :])
        nc.sync.dma_start(
            out=xaug[0:1, :], in_=xt_scr[:, :].rearrange("p c -> (p c)")[None, :]
        )

        # Stationary aug tiles:
        #   AUG   [2, 2H]: (wi*64, bsum*64) rows for r,z; rhs = xaug (x, 1)
        #   AUGNB [1, H]:  (b_hh_n*64); rhs = ones1
        #   AUGG  [2, H]:  (wi_n, bih_n) UNSCALED for psn2 = gx_n; rhs = xaug
        AUG = cpool.tile([2, 2 * H], BF16)
        AUGNB = cpool.tile([1, H], BF16)
        AUGG = cpool.tile([2, H], BF16)

        def row_dma(q, dst, r, kind, p0, p1):
            q.dma_start(
                out=dst[r : r + 1, :],
                in_=scr_d[kind, p0:p1, :].rearrange("p c -> (p c)")[None, :],
            )

        # kinds: 0 = wi*64, 1 = bs*64 (b_hh_n*64 on rows 8:12), 2 = bih, 3 = wi
        row_dma(nc.sync, AUG, 0, 0, 0, 8)
        row_dma(nc.sync, AUG, 1, 1, 0, 8)
        row_dma(nc.scalar, AUGNB, 0, 1, 8, 12)
        row_dma(nc.scalar, AUGG, 0, 3, 8, 12)
        row_dma(nc.scalar, AUGG, 1, 2, 8, 12)

        # w_hh.T in fp8, x64, DoubleRow-packed:
        #   block bb = c*2 + half  (c = global j-chunk 0..11, half = k-half)
        #   wT8[p, bb*256 + i*128 + j] = 64 * w_hh[c*128+j, (2*half+i)*128 + p]
        wT8 = cpool.tile([128, 24 * 256], FP8)
        prep_tags = ["psR0", "psZ0", "psN0", "psR1", "psZ1", "psN1"]
        pi = 0
        for c in range(12):
            for k in range(NK):
                tp = ppool.tile([128, 128], F32, tag=prep_tags[pi % 6], bufs=1,
                                name=f"wprep_{c}_{k}")
                nc.tensor.transpose(
                    tp[:, :], wstage[:, c * H + k * 128 : c * H + (k + 1) * 128],
                    ident128,
                )
                bb = c * 2 + k // 2
                dst = wT8[:, bb * 256 + (k % 2) * 128 : bb * 256 + (k % 2) * 128 + 128]
                if pi % 2 == 0:
                    nc.vector.tensor_scalar_mul(dst, tp[:, :], WSC)
                else:
                    nc.scalar.activation(dst, tp[:, :], AF.Copy, scale=WSC)
                pi += 1

        # state init: u8/q8 fp8 (PE inputs), s bf16 (master h), per wave
        u8T, q8T, sT = [], [], []
        for w in range(W):
            u0 = spool.tile([128, NK * BW], FP8, tag=f"u8{w}", name=f"u8{w}_init")
            nc.gpsimd.memset(u0[:, :], 0.0)
            u8T.append(u0)
            q0 = spool.tile([128, NK * BW], FP8, tag=f"q8{w}", name=f"q8{w}_init")
            nc.gpsimd.memset(q0[:, :], 0.0)
            q8T.append(q0)
            s0 = spool.tile([128, NK * BW], BF16, tag=f"s{w}", name=f"s{w}_init")
            nc.gpsimd.memset(s0[:, :], 0.0)
            sT.append(s0)

        def drview(tile_, half):
            # [128, 2, BW] moving view: k-tiles (2*half, 2*half+1)
            return tile_[:, half * 2 * BW : (half + 1) * 2 * BW].rearrange(
                "p (i b) -> p i b", i=2
            )

        GBASE = {0: 0, 1: 4, 2: 8}  # gate -> global j-chunk base (r, z, n)

        def emit_pe_early(w, t, ps, q8t, gates=(0, 2, 1)):
            # Open one accumulation group per gate bank: augs + q8 DRs for all
            # four j-chunks.  start=True only on the first matmul of the bank
            # (ZERO_REGION covers the whole bank; later writes zero via the
            # pending mark).  The group stays open; u8 DRs close it later.
            psr, psz, psn, psn2 = ps
            xs = xaug[0:2, t * B + w * BW : t * B + (w + 1) * BW]
            os = ones1[0:1, w * BW : (w + 1) * BW]
            for g in gates:  # r, n, z; augs first (xaug-gated, always
                # ready) so only the 8 q8-gated DRs per gate can ever sit
                # ahead of the other wave's chain-critical u8 DRs
                pst = (psr, psz, psn)[g]
                for jc in range(NJ):
                    po = pst[:, jc * BW : (jc + 1) * BW]
                    if g < 2:
                        nc.tensor.matmul(
                            po, AUG[0:2, g * H + jc * 128 : g * H + (jc + 1) * 128],
                            xs, start=(jc == 0), stop=False,
                        )
                    else:
                        nc.tensor.matmul(
                            po, AUGNB[0:1, jc * 128 : (jc + 1) * 128],
                            os, start=(jc == 0), stop=False,
                        )
                for jc in range(NJ):
                    po = pst[:, jc * BW : (jc + 1) * BW]
                    c = GBASE[g] + jc
                    for half in range(2):
                        bb = c * 2 + half
                        nc.tensor.matmul(
                            po,
                            wT8[:, bb * 256 : (bb + 1) * 256].rearrange(
                                "p (i j) -> p i j", i=2
                            ),
                            drview(q8t, half),
                            start=False, stop=False,
                            perf_mode=DR,
                        )

        def emit_pe_late(w, t, ps, u8t, gates=(0, 2, 1)):
            # u8-gated DRs close each gate bank's group (r first).
            psr, psz, psn, psn2 = ps
            for g in gates:
                pst = (psr, psz, psn)[g]
                for jc in range(NJ):
                    po = pst[:, jc * BW : (jc + 1) * BW]
                    c = GBASE[g] + jc
                    for half in range(2):
                        bb = c * 2 + half
                        nc.tensor.matmul(
                            po,
                            wT8[:, bb * 256 : (bb + 1) * 256].rearrange(
                                "p (i j) -> p i j", i=2
                            ),
                            drview(u8t, half),
                            start=False,
                            stop=(jc == NJ - 1 and half == 1),
                            perf_mode=DR,
                        )

        def emit_psn2_augs(w, t, ps):
            # gx_n into psn2; first part of the single psn2 group (identity
            # matmul with m closes it in emit_gpairs)
            psr, psz, psn, psn2 = ps
            xs = xaug[0:2, t * B + w * BW : t * B + (w + 1) * BW]
            for jc in range(NJ):
                nc.tensor.matmul(
                    psn2[:, jc * BW : (jc + 1) * BW],
                    AUGG[0:2, jc * 128 : (jc + 1) * 128],
                    xs, start=(jc == 0), stop=False,
                )

        def emit_gpairs(w, t, ps, st):
            psr, psz, psn, psn2 = ps
            nc.tensor.matmul(
                psn2[:, 0 : 4 * BW], identb[:, :], st["m"][:, 0 : 4 * BW],
                start=False, stop=True,
            )

        def emit_sigr(w, t, ps, st):
            psr, psz, psn, psn2 = ps
            if st.get("rz_t") != t:
                st["rz"] = wpool.tile([128, 8 * BW], BF16, tag=f"rz{w}",
                                      name=f"rz{w}_{t}")
                st["rz_t"] = t
            nc.scalar.activation(st["rz"][:, 0 : 4 * BW], psr[:, :], AF.Sigmoid,
                                 scale=1.0 / WSC)

        def emit_sigz(w, t, ps, st):
            psr, psz, psn, psn2 = ps
            if st.get("rz_t") != t:
                st["rz"] = wpool.tile([128, 8 * BW], BF16, tag=f"rz{w}",
                                      name=f"rz{w}_{t}")
                st["rz_t"] = t
            nc.scalar.activation(st["rz"][:, 4 * BW : 8 * BW], psz[:, :], AF.Sigmoid,
                                 scale=1.0 / WSC)

        def emit_m(w, t, ps, st):
            # m = (P_n/64) * r stays on DVE: GPSIMD cannot read PSUM on
            # real hardware (compile fails), even though the cost model
            # would price it lower there
            psr, psz, psn, psn2 = ps
            rz = st["rz"]
            m = wpool.tile([128, 4 * BW], BF16, tag=f"m{w}", name=f"m{w}_{t}")
            nc.vector.scalar_tensor_tensor(
                m[:, :], psn[:, :], 1.0 / WSC, rz[:, 0 : 4 * BW],
                ALU.mult, ALU.mult,
            )
            st["m"] = m

        def emit_cv(w, t, st):
            rz = st["rz"]
            cv = wpool.tile([128, 4 * BW], BF16, tag=f"cv{w}", name=f"cv{w}_{t}")
            nc.vector.tensor_scalar(cv[:, :], rz[:, 4 * BW : 8 * BW], 1.0, -1.0,
                                    ALU.subtract, ALU.mult)
            st["cv"] = cv

        def emit_q8(w, t, st):
            # q8 = fp8(z * s_old) for PE; both waves' q8 run on Pool BEFORE
            # the q16s so next step's early matmuls unblock as soon as
            # possible
            rz = st["rz"]
            q8n = spool.tile([128, NK * BW], FP8, tag=f"q8{w}", name=f"q8{w}_{t}")
            nc.gpsimd.tensor_tensor(q8n[:, :], rz[:, 4 * BW : 8 * BW], sT[w][:, :],
                                    ALU.mult)
            st["q8n"] = q8n

        def emit_q16(w, t, st, eng=None):
            # q16 = bf16 version for the exact master state.  Wave 0's runs
            # on DVE (2x mode); wave 1's runs on Pool BEFORE q8_1, which
            # delays q8_1 just enough that next step's w1-earlies are
            # statically ordered AFTER the chain-critical w0 lates.
            rz = st["rz"]
            q16 = wpool.tile([128, 4 * BW], BF16, tag=f"q16{w}", name=f"q16{w}_{t}")
            (eng or nc.vector).tensor_tensor(q16[:, :], rz[:, 4 * BW : 8 * BW],
                                             sT[w][:, :], ALU.mult)
            st["q16"] = q16

        def emit_tanh(w, t, ps, st):
            psr, psz, psn, psn2 = ps
            n = wpool.tile([128, 4 * BW], BF16, tag=f"n{w}", name=f"n{w}_{t}")
            nc.scalar.activation(n[:, :], psn2[:, :], AF.Tanh)
            st["n"] = n

        def emit_tail_u(w, t, st):
            n, cv = st["n"], st["cv"]
            # u8 first: it is the ONLY op the next step's matmuls wait on
            u8n = spool.tile([128, NK * BW], FP8, tag=f"u8{w}", name=f"u8{w}_{t}")
            nc.vector.tensor_tensor(u8n[:, :], cv[:, :], n[:, :], ALU.mult)
            u16 = wpool.tile([128, 4 * BW], BF16, tag=f"u16{w}", name=f"u16{w}_{t}")
            nc.vector.tensor_tensor(u16[:, :], cv[:, :], n[:, :], ALU.mult)
            st["u16"] = u16
            u8T[w] = u8n

        def emit_s(w, t, st):
            sn = spool.tile([128, NK * BW], BF16, tag=f"s{w}", name=f"s{w}_{t}")
            nc.vector.tensor_tensor(sn[:, :], st["u16"][:, :], st["q16"][:, :],
                                    ALU.add)
            sT[w] = sn

        def alloc_ps(t):
            out = []
            for w in range(W):
                psr = ppool.tile([128, 4 * BW], F32, tag=f"psR{w}", bufs=1,
                                 name=f"psr{w}_{t}")
                psz = ppool.tile([128, 4 * BW], F32, tag=f"psZ{w}", bufs=1,
                                 name=f"psz{w}_{t}")
                psn = ppool.tile([128, 4 * BW], F32, tag=f"psN{w}", bufs=1,
                                 name=f"psn{w}_{t}")
                psn2 = ppool.tile([128, 4 * BW], F32, tag="psPREP", bufs=2,
                                  name=f"psn2_{w}_{t}")
                out.append((psr, psz, psn, psn2))
            return out

        # ---- the recurrence, fully unrolled, 2 waves interleaved ----
        # PE order per step: earlies(0) [prev iter] | late0 augs0 earlies1
        # late1 ... ident0 augs1 earlies0(t+1) ident1
        sts = [{}, {}]
        pss = alloc_ps(0)
        emit_pe_early(0, 0, pss[0], q8T[0])
        for t in range(T):
            pss_next = alloc_ps(t + 1) if t + 1 < T else None
            emit_pe_early(1, t, pss[1], q8T[1], gates=(0,))
            with tc.high_priority():
                emit_pe_late(0, t, pss[0], u8T[0])
            emit_psn2_augs(0, t, pss[0])
            # n/z earlies of wave 1 sit AFTER the chain-critical w0 lates in
            # priority so their drain cannot delay sigmoid(r0)
            emit_pe_early(1, t, pss[1], q8T[1], gates=(2, 1))
            emit_pe_late(1, t, pss[1], u8T[1])
            with tc.high_priority():
                emit_sigr(0, t, pss[0], sts[0])
            emit_sigz(0, t, pss[0], sts[0])
            with tc.high_priority():
                emit_m(0, t, pss[0], sts[0])
            emit_cv(0, t, sts[0])
            emit_q16(0, t, sts[0])
            emit_q8(0, t, sts[0])
            emit_sigr(1, t, pss[1], sts[1])
            with tc.high_priority():
                emit_gpairs(0, t, pss[0], sts[0])
                emit_tanh(0, t, pss[0], sts[0])
            emit_sigz(1, t, pss[1], sts[1])
            emit_m(1, t, pss[1], sts[1])
            emit_q8(1, t, sts[1])
            emit_psn2_augs(1, t, pss[1])
            if pss_next is not None:
                emit_pe_early(0, t + 1, pss_next[0], sts[0]["q8n"])
            with tc.high_priority():
                emit_tail_u(0, t, sts[0])
            emit_cv(1, t, sts[1])
            emit_q16(1, t, sts[1])
            emit_gpairs(1, t, pss[1], sts[1])
            emit_tanh(1, t, pss[1], sts[1])
            emit_tail_u(1, t, sts[1])
            emit_s(0, t, sts[0])
            emit_s(1, t, sts[1])
            q8T[0] = sts[0]["q8n"]
            q8T[1] = sts[1]["q8n"]
            pss = pss_next

        # ---- head: out = relu(h) @ fc_w.T + fc_b ----
        pso = ppool.tile([B, 1], F32, tag="psPREP", bufs=2, name="ps_fc")
        for w in range(W):
            reluh = wpool.tile([128, NK * BW], F32, tag=f"relu{w}", name=f"relu{w}")
            nc.scalar.activation(reluh[:, :], sT[w][:, :], AF.Relu)
            po = pso[w * BW : (w + 1) * BW, :]
            nc.tensor.matmul(
                po, onesf[:, 0:BW], fcbf[0:1, 0:1], start=True, stop=False
            )
            for k in range(NK):
                nc.tensor.matmul(
                    po,
                    reluh[:, k * BW : (k + 1) * BW],
                    fcwf[:, k : k + 1],
                    start=False, stop=(k == NK - 1),
                )
        outw = wpool.tile([B, 1], F32, tag="outw", name="out_sb")
        nc.vector.tensor_copy(outw[:, :], pso[:, :])
        nc.sync.dma_start(out=out_d[:, :], in_=outw[:, :])


_NC_CACHE: dict[int, bass.Bass] = {}


def _get_nc(T: int = T_FULL) -> bass.Bass:
    if T not in _NC_CACHE:
        _NC_CACHE[T] = build_nc(T)
    return _NC_CACHE[T]


def kernel(x, w_ih, w_hh, b_ih, b_hh, fc_w, fc_b, _trace=False, _tmpdir=None):
    x = np.ascontiguousarray(np.asarray(x, dtype=np.float32))
    nc = _get_nc(x.shape[1])
    shared = {
        "w_hh": np.ascontiguousarray(np.asarray(w_hh, np.float32)),
        "w_ih": np.ascontiguousarray(np.asarray(w_ih, np.float32)),
        "b_ih": np.ascontiguousarray(np.asarray(b_ih, np.float32)),
        "b_hh": np.ascontiguousarray(np.asarray(b_hh, np.float32)),
        "fc_w": np.ascontiguousarray(np.asarray(fc_w, np.float32)),
        "fc_b": np.ascontiguousarray(np.asarray(fc_b, np.float32)),
    }
    in_maps = [{"x": x[c * B : (c + 1) * B], **shared} for c in range(N_CORES)]
    res = run_bass_kernel_spmd(
        nc, in_maps, list(range(N_CORES)), trace=_trace, tmpdir=_tmpdir
    )
    out = np.concatenate([res.results[c]["out"] for c in range(N_CORES)], axis=0)
    if _trace:
        return out, res
    return out
